# revision 1
# baseline (speedup 1.0000x reference)
"""Self-contained Trainium2 Bass kernel for sparse attention.

Sharding: 8 cores = (image b, L-half). Each core receives its image's x0
ROTATED so its own 4096 rows come first (gather indices are remapped on
the host to match). The core computes LN+K/V for all 8192 rows, writes
packed bf16 [k|v] rows to DRAM scratch, then per 128-row tile gathers
2048 neighbor rows with dma_gather and runs attention + merge + MLP +
LN2 fully on-chip. No collectives.
"""
import numpy as np
import ml_dtypes

import concourse.bass as bass
import concourse.tile as tile
from concourse import bacc, library_config, mybir

F32 = mybir.dt.float32
BF16 = mybir.dt.bfloat16
I16 = mybir.dt.int16
I32 = mybir.dt.int32
AX = mybir.AxisListType
OP = mybir.AluOpType
AF = mybir.ActivationFunctionType
ts = bass.ts

L, C, NJ, NH, HD = 8192, 128, 16, 8, 16
LH = L // 2            # rows computed per core
NT_FULL = L // 128     # 64 k/v tiles
NT_HALF = LH // 128    # 32 attention tiles
EPS = 1e-5


NSWQ = int(__import__("os").environ.get("NSWQ", "1"))


def build_nc(nontrivial_ln1: bool, nontrivial_ln2: bool):
    nc = bacc.Bacc(None, target_bir_lowering=False, debug=False,
                   num_swdge_queues=NSWQ)

    x0f = nc.declare_dram_parameter("x0f", [L, C], F32, isOutput=False)
    gidx32 = nc.declare_dram_parameter("gidx32", [128, NT_HALF * NJ], I32, isOutput=False)
    wnames = ["wm", "w1", "w2"]
    wparams = {n: nc.declare_dram_parameter(n, [C, C], BF16, isOutput=False) for n in wnames}
    wkvp = nc.declare_dram_parameter("wqkv", [C, 3 * C], BF16, isOutput=False)
    identp = nc.declare_dram_parameter("ident", [C, C], BF16, isOutput=False)
    if nontrivial_ln1:
        bqkv = nc.declare_dram_parameter("bqkv", [1, 3 * C], F32, isOutput=False)
    if nontrivial_ln2:
        g2b2 = nc.declare_dram_parameter("g2b2", [1, 2 * C], F32, isOutput=False)
    out = nc.declare_dram_parameter("out", [LH, C], F32, isOutput=True)

    with tile.TileContext(nc) as tc:
        with (
            tc.tile_pool(name="res", bufs=1) as res,
            tc.tile_pool(name="dram", bufs=1, space="DRAM") as dram,
        ):
            kv_dram = dram.tile([L, 2 * C], BF16)
            x0_all = res.tile([128, NT_FULL * 128], F32)   # all x0 tiles (ours first)
            x0_res = x0_all  # our-half rows are tiles 0..NT_HALF-1
            q_res = res.tile([128, NT_HALF * 128], BF16)   # our-half q tiles
            m2_all = res.tile([128, NT_HALF * 128], F32)   # MLP outputs for batched LN2
            muA = res.tile([128, NT_FULL], F32)
            rstdA = res.tile([128, NT_FULL], F32)
            idx32_res = res.tile([128, NT_HALF * NJ], I32)
            nc.sync.dma_start(idx32_res[:], gidx32[:])
            ident = res.tile([128, 128], BF16)
            wsb = {n: res.tile([C, C], BF16, name=f"w_{n}", tag=f"w_{n}") for n in wnames}
            wqkv_sb = res.tile([C, 3 * C], BF16)
            nc.sync.dma_start(wqkv_sb[:], wkvp[:])

            for n in wnames:
                nc.sync.dma_start(wsb[n][:], wparams[n][:])
            nc.sync.dma_start(ident[:], identp[:])
            if nontrivial_ln1:
                bqkv_sb = res.tile([1, 3 * C], F32)
                nc.sync.dma_start(bqkv_sb[:], bqkv[:])
            if nontrivial_ln2:
                g2b2_sb = res.tile([1, 2 * C], F32)
                nc.sync.dma_start(g2b2_sb[:], g2b2[:])

            # ---------------- Phase 1a: load x0 + batched LN1 stats ----------------
            NCH = 8                      # tiles per stats chunk
            with tc.tile_pool(name="pa", bufs=2) as pa:
                ssumA = res.tile([128, NT_FULL], F32)
                s2A = res.tile([128, NT_FULL], F32)
                varA = res.tile([128, NT_FULL], F32)
                stdA = res.tile([128, NT_FULL], F32)
                mu2A = res.tile([128, NT_FULL], F32)
                for ch in range(NT_FULL // NCH):
                    lo = ch * NCH * 128
                    cs = slice(ch * NCH, (ch + 1) * NCH)
                    nc.sync.dma_start(
                        x0_all[:, lo:lo + NCH * 128].rearrange("p (t c) -> p t c", t=NCH),
                        x0f[ch * NCH * 128:(ch + 1) * NCH * 128, :].rearrange(
                            "(t p) c -> p t c", p=128))
                    sqc = pa.tile([128, NCH * 128], F32, tag="sqch")
                    nc.scalar.activation(sqc[:], x0_all[:, lo:lo + NCH * 128], AF.Square)
                    nc.vector.tensor_reduce(
                        ssumA[:, cs],
                        x0_all[:, lo:lo + NCH * 128].rearrange("p (t c) -> p t c", t=NCH),
                        axis=AX.X, op=OP.add)
                    nc.vector.tensor_reduce(
                        s2A[:, cs],
                        sqc[:].rearrange("p (t c) -> p t c", t=NCH),
                        axis=AX.X, op=OP.add)
                    # finalize this chunk's mu/rstd now so phase 1b can start
                    # on chunk 0 without waiting for the whole stats pass
                    nc.vector.tensor_scalar_mul(muA[:, cs], ssumA[:, cs], 1.0 / C)
                    nc.vector.tensor_tensor(mu2A[:, cs], muA[:, cs], muA[:, cs], op=OP.mult)
                    nc.vector.tensor_scalar(
                        varA[:, cs], s2A[:, cs], scalar1=1.0 / C, scalar2=EPS,
                        op0=OP.mult, op1=OP.add)
                    nc.vector.tensor_tensor(varA[:, cs], varA[:, cs], mu2A[:, cs], op=OP.subtract)
                    nc.scalar.activation(stdA[:, cs], varA[:, cs], AF.Sqrt)
                    nc.vector.reciprocal(rstdA[:, cs], stdA[:, cs])

            # ---------------- Phase 1b: xn + K/V (+Q) projections ----------------
            with (
                tc.tile_pool(name="p1", bufs=3) as p1,
                tc.tile_pool(name="p1s", bufs=2) as p1s,
                tc.tile_pool(name="ps1", bufs=2, space="PSUM") as ps1,
            ):
                for t in range(NT_FULL):
                    ours = t < NT_HALF
                    x0t = x0_all[:, ts(t, 128)]
                    xn = p1.tile([128, 128], BF16, tag="xn")
                    nc.vector.tensor_scalar(
                        xn[:], x0t, scalar1=muA[:, t:t + 1], scalar2=rstdA[:, t:t + 1],
                        op0=OP.subtract, op1=OP.mult,
                    )

                    # transpose xn -> xnT (bf16)
                    xnT_ps = ps1.tile([128, 128], BF16, tag="xnT_ps")
                    xnT = p1.tile([128, 128], BF16, tag="xnT")
                    nc.tensor.transpose(xnT_ps[:], xn[:], ident[:])
                    nc.scalar.copy(xnT[:], xnT_ps[:])

                    # q/k/v projections in one matmul -> packed [k|v] bf16 rows;
                    # 8 tiles of rows accumulate in SBUF, written with one DMA.
                    if t % 8 == 0:
                        kvt8 = p1.tile([128, 8 * 2 * C], BF16, tag="kvt8")
                    kvt = kvt8[:, (t % 8) * 2 * C:(t % 8 + 1) * 2 * C]
                    if ours:
                        qkv_ps = ps1.tile([128, 3 * C], F32, tag="qkv_ps")
                        nc.tensor.matmul(qkv_ps[:], lhsT=xnT[:], rhs=wqkv_sb[:], start=True, stop=True)
                        kv_ps = qkv_ps[:, C:3 * C]
                        if nontrivial_ln1:
                            nc.vector.tensor_tensor(
                                q_res[:, ts(t, 128)], qkv_ps[:, 0:C],
                                bqkv_sb[:, 0:C].to_broadcast([128, C]), op=OP.add)
                        else:
                            nc.scalar.copy(q_res[:, ts(t, 128)], qkv_ps[:, 0:C])
                    else:
                        kv_ps2 = ps1.tile([128, 2 * C], F32, tag="kv_ps")
                        nc.tensor.matmul(kv_ps2[:], lhsT=xnT[:], rhs=wqkv_sb[:, C:3 * C], start=True, stop=True)
                        kv_ps = kv_ps2[:]
                    if nontrivial_ln1:
                        nc.vector.tensor_tensor(
                            kvt, kv_ps,
                            bqkv_sb[:, C:3 * C].to_broadcast([128, 2 * C]), op=OP.add)
                    elif t % 2 == 0:
                        nc.scalar.copy(kvt[:, 0:C], kv_ps[:, 0:C])
                        nc.vector.tensor_scalar_mul(kvt[:, C:2 * C], kv_ps[:, C:2 * C], 1.0)
                    else:
                        nc.vector.tensor_scalar_mul(kvt, kv_ps, 1.0)
                    if t % 8 == 7:
                        nc.sync.dma_start(
                            kv_dram[(t - 7) * 128:(t + 1) * 128, :].rearrange(
                                "(u p) x -> p u x", p=128),
                            kvt8[:].rearrange("p (u x) -> p u x", u=8))

            # ---------------- Phase 2: gather + attention + MLP ----------------
            with (
                tc.tile_pool(name="p2", bufs=6) as p2,
                tc.tile_pool(name="p2s", bufs=3) as p2s,
                tc.tile_pool(name="ps2", bufs=1, space="PSUM") as ps2,
            ):
                for t in range(NT_HALF):
                    kvg = p2.tile([128, NJ * 2 * C], BF16, tag="kvg")
                    for j in range(NJ):
                        gi = nc.gpsimd.indirect_dma_start(
                            out=kvg[:, ts(j, 2 * C)],
                            out_offset=None,
                            in_=kv_dram[:],
                            in_offset=bass.IndirectOffsetOnAxis(
                                ap=idx32_res[:, t * NJ + j:t * NJ + j + 1], axis=0),
                        )
                        qn = (t * NJ + j) % NSWQ
                        if qn:
                            gi.ins.queue = f"qPoolDynamic{qn}"
                    kvg_j = kvg[:].rearrange("p (j x) -> p j x", j=NJ)

                    # qk = sum_d q*kg per (j, head)
                    prod = p2s.tile([128, NJ * C], BF16, tag="prod")
                    qk = p2s.tile([128, NJ * NH], F32, tag="qk")
                    nc.vector.tensor_tensor(
                        prod[:].rearrange("p (j c) -> p j c", j=NJ),
                        q_res[:, ts(t, 128)].unsqueeze(1).to_broadcast([128, NJ, C]),
                        kvg_j[:, :, 0:C],
                        op=OP.mult,
                    )
                    tr8 = p2s.tile([128, NJ * NH * 8], BF16, tag="tr8")
                    tr4 = p2s.tile([128, NJ * NH * 4], BF16, tag="tr4")
                    tr2 = p2s.tile([128, NJ * NH * 2], BF16, tag="tr2")
                    p4d = prod[:].rearrange("p (j h d) -> p j h d", j=NJ, h=NH)
                    t8 = tr8[:].rearrange("p (j h d) -> p j h d", j=NJ, h=NH)
                    t4 = tr4[:].rearrange("p (j h d) -> p j h d", j=NJ, h=NH)
                    t2 = tr2[:].rearrange("p (j h d) -> p j h d", j=NJ, h=NH)
                    nc.vector.tensor_tensor(t8, p4d[:, :, :, 0:8], p4d[:, :, :, 8:16], op=OP.add)
                    nc.vector.tensor_tensor(t4, t8[:, :, :, 0:4], t8[:, :, :, 4:8], op=OP.add)
                    nc.vector.tensor_tensor(t2, t4[:, :, :, 0:2], t4[:, :, :, 2:4], op=OP.add)
                    nc.vector.tensor_tensor(
                        qk[:].rearrange("p (j h) -> p j h", j=NJ, h=NH).unsqueeze(3),
                        t2[:, :, :, 0:1], t2[:, :, :, 1:2], op=OP.add)
                    # softmax over j (no max subtraction; |qk| <~ 6)
                    E = p2s.tile([128, NJ * NH], BF16, tag="E")
                    sE = p2s.tile([128, NH], F32, tag="sE")
                    rec = p2s.tile([128, NH], F32, tag="rec")
                    A = p2s.tile([128, NJ * NH], BF16, tag="A")
                    nc.scalar.activation(E[:], qk[:], AF.Exp)
                    nc.vector.tensor_reduce(
                        sE[:], E[:].rearrange("p (j h) -> p h j", j=NJ), axis=AX.X, op=OP.add
                    )
                    nc.vector.reciprocal(rec[:], sE[:])
                    nc.vector.tensor_tensor(
                        A[:].rearrange("p (j h) -> p j h", j=NJ),
                        E[:].rearrange("p (j h) -> p j h", j=NJ),
                        rec[:].unsqueeze(1).to_broadcast([128, NJ, NH]),
                        op=OP.mult,
                    )
                    # att = sum_j A * vg  (A broadcast over d fused into the multiply)
                    prod2 = p2s.tile([128, NJ * C], BF16, tag="prod2")
                    att = p2s.tile([128, C], BF16, tag="att")
                    nc.vector.tensor_tensor(
                        prod2[:].rearrange("p (j h d) -> p j h d", j=NJ, h=NH),
                        kvg_j[:, :, C:2 * C].rearrange("p j (h d) -> p j h d", h=NH),
                        A[:].rearrange("p (j h) -> p j h", j=NJ).unsqueeze(3).to_broadcast([128, NJ, NH, HD]),
                        op=OP.mult,
                    )
                    av8 = p2s.tile([128, 8 * C], BF16, tag="av8")
                    av4 = p2s.tile([128, 4 * C], BF16, tag="av4")
                    av2 = p2s.tile([128, 2 * C], BF16, tag="av2")
                    nc.vector.tensor_tensor(av8[:], prod2[:, 0:8 * C], prod2[:, 8 * C:16 * C], op=OP.add)
                    nc.vector.tensor_tensor(av4[:], av8[:, 0:4 * C], av8[:, 4 * C:8 * C], op=OP.add)
                    nc.vector.tensor_tensor(av2[:], av4[:, 0:2 * C], av4[:, 2 * C:4 * C], op=OP.add)
                    nc.vector.tensor_tensor(att[:], av2[:, 0:C], av2[:, C:2 * C], op=OP.add)

                    # merge: qv = att @ Wm.T ; message = x0 + qv
                    attT_ps = ps2.tile([128, 128], BF16, tag="attT_ps")
                    attT = p2s.tile([128, 128], BF16, tag="attT")
                    nc.tensor.transpose(attT_ps[:], att[:], ident[:])
                    nc.scalar.copy(attT[:], attT_ps[:])
                    qv_ps = ps2.tile([128, 128], F32, tag="qv_ps")
                    nc.tensor.matmul(qv_ps[:], lhsT=attT[:], rhs=wsb["wm"][:], start=True, stop=True)
                    msg = p2s.tile([128, 128], BF16, tag="msg")
                    nc.vector.tensor_tensor(msg[:], x0_res[:, ts(t, 128)], qv_ps[:], op=OP.add)

                    # mlp
                    msgT_ps = ps2.tile([128, 128], BF16, tag="msgT_ps")
                    msgT = p2s.tile([128, 128], BF16, tag="msgT")
                    nc.tensor.transpose(msgT_ps[:], msg[:], ident[:])
                    nc.scalar.copy(msgT[:], msgT_ps[:])
                    m1_ps = ps2.tile([128, 128], F32, tag="m1_ps")
                    nc.tensor.matmul(m1_ps[:], lhsT=msgT[:], rhs=wsb["w1"][:], start=True, stop=True)
                    m1 = p2s.tile([128, 128], BF16, tag="m1")
                    nc.scalar.activation(m1[:], m1_ps[:], AF.Relu)
                    m1T_ps = ps2.tile([128, 128], BF16, tag="m1T_ps")
                    m1T = p2s.tile([128, 128], BF16, tag="m1T")
                    nc.tensor.transpose(m1T_ps[:], m1[:], ident[:])
                    nc.scalar.copy(m1T[:], m1T_ps[:])
                    m2_ps = ps2.tile([128, 128], F32, tag="m2_ps")
                    nc.tensor.matmul(m2_ps[:], lhsT=m1T[:], rhs=wsb["w2"][:], start=True, stop=True)
                    nc.scalar.activation(m2_all[:, ts(t, 128)], m2_ps[:], AF.Copy)

                    # ---- LN2 + residual, finalized per 8-tile chunk so it
                    # overlaps the gather stream instead of trailing it ----
                    FCH = 8
                    if t % FCH == FCH - 1:
                        c0 = t - (FCH - 1)
                        m2c = m2_all[:, c0 * 128:(t + 1) * 128]
                        sqf = p2s.tile([128, FCH * 128], F32, tag="sqf")
                        ssumf = p2s.tile([128, FCH], F32, tag="ssumf")
                        s2f = p2s.tile([128, FCH], F32, tag="s2f")
                        muf = p2s.tile([128, FCH], F32, tag="muf")
                        mu2f = p2s.tile([128, FCH], F32, tag="mu2f")
                        varf = p2s.tile([128, FCH], F32, tag="varf")
                        stdf = p2s.tile([128, FCH], F32, tag="stdf")
                        rstdf = p2s.tile([128, FCH], F32, tag="rstdf")
                        nc.scalar.activation(sqf[:], m2c, AF.Square)
                        nc.vector.tensor_reduce(
                            ssumf[:], m2c.rearrange("p (t c) -> p t c", t=FCH),
                            axis=AX.X, op=OP.add)
                        nc.vector.tensor_reduce(
                            s2f[:], sqf[:].rearrange("p (t c) -> p t c", t=FCH),
                            axis=AX.X, op=OP.add)
                        nc.vector.tensor_scalar_mul(muf[:], ssumf[:], 1.0 / C)
                        nc.vector.tensor_tensor(mu2f[:], muf[:], muf[:], op=OP.mult)
                        nc.vector.tensor_scalar(
                            varf[:], s2f[:], scalar1=1.0 / C, scalar2=EPS,
                            op0=OP.mult, op1=OP.add)
                        nc.vector.tensor_tensor(varf[:], varf[:], mu2f[:], op=OP.subtract)
                        nc.scalar.activation(stdf[:], varf[:], AF.Sqrt)
                        nc.vector.reciprocal(rstdf[:], stdf[:])
                        for u in range(FCH):
                            tt = c0 + u
                            outt = p2s.tile([128, 128], F32, tag="outt")
                            nc.vector.tensor_scalar(
                                outt[:], m2_all[:, ts(tt, 128)],
                                scalar1=muf[:, u:u + 1], scalar2=rstdf[:, u:u + 1],
                                op0=OP.subtract, op1=OP.mult)
                            if nontrivial_ln2:
                                nc.vector.tensor_tensor(
                                    outt[:], outt[:], g2b2_sb[:, 0:C].to_broadcast([128, C]), op=OP.mult)
                                nc.vector.tensor_tensor(
                                    outt[:], outt[:], g2b2_sb[:, C:2 * C].to_broadcast([128, C]), op=OP.add)
                            nc.vector.tensor_tensor(outt[:], outt[:], x0_res[:, ts(tt, 128)], op=OP.add)
                            nc.sync.dma_start(out[ts(tt, 128), :], outt[:])

    nc.finalize()
    return nc


def prep_core_inputs(x0_img: np.ndarray, query_img: np.ndarray, half: int, w):
    """Host-side prep for one core. w: dict of raw f32 weights g1,b1,g2,b2,Wq..W2."""
    ofs = half * LH
    x0r = np.ascontiguousarray(np.roll(x0_img, -ofs, axis=0))
    lq = query_img[ofs:ofs + LH, :].astype(np.int64)
    lq = (lq - ofs) % L  # remap into rotated coordinates

    gidx32 = np.zeros((128, NT_HALF * NJ), np.int32)
    for t in range(NT_HALF):
        gidx32[:, t * NJ:(t + 1) * NJ] = lq[t * 128:(t + 1) * 128, :]
    bf = ml_dtypes.bfloat16
    g1 = w["g1"]
    m = {
        "x0f": x0r,
        "gidx32": gidx32,
        "ident": np.eye(C, dtype=np.float32).astype(bf),
        "wqkv": np.concatenate(
            [np.ascontiguousarray((0.25 * w["Wq"] * g1[None, :]).T),
             np.ascontiguousarray((w["Wk"] * g1[None, :]).T),
             np.ascontiguousarray((w["Wv"] * g1[None, :]).T)], axis=1).astype(bf),
        "wm": np.ascontiguousarray(w["Wm"].T).astype(bf),
        "w1": np.ascontiguousarray(w["W1"].T).astype(bf),
        "w2": np.ascontiguousarray(w["W2"].T).astype(bf),
    }
    b1 = w["b1"]
    nontrivial_ln1 = bool(np.any(b1 != 0.0))
    if nontrivial_ln1:
        m["bqkv"] = np.concatenate(
            [0.25 * (w["Wq"] @ b1), w["Wk"] @ b1, w["Wv"] @ b1]
        ).reshape(1, 3 * C).astype(np.float32)
    nontrivial_ln2 = bool(np.any(w["g2"] != 1.0) or np.any(w["b2"] != 0.0))
    if nontrivial_ln2:
        m["g2b2"] = np.concatenate([w["g2"], w["b2"]]).reshape(1, 2 * C).astype(np.float32)
    return m, nontrivial_ln1, nontrivial_ln2


def kernel(**inputs):
    from concourse.bass_utils import run_bass_kernel_spmd

    x0 = np.asarray(inputs["x0"], np.float32)
    query = np.asarray(inputs["query"])
    w = {k: np.asarray(inputs[k], np.float32)
         for k in ["Wq", "Wk", "Wv", "Wm", "W1", "W2", "g1", "b1", "g2", "b2"]}
    B = x0.shape[0]

    in_maps = []
    nt1 = nt2 = False
    for c in range(8):
        b, half = c // 2, c % 2
        m, nt1, nt2 = prep_core_inputs(x0[b], np.asarray(query[b]), half, w)
        in_maps.append(m)

    nc = build_nc(nt1, nt2)
    res = run_bass_kernel_spmd(nc, in_maps, core_ids=list(range(8)))

    outp = np.empty((B, L, C), np.float32)
    for c in range(8):
        b, half = c // 2, c % 2
        outp[b, half * LH:(half + 1) * LH, :] = res.results[c]["out"]
    return outp



# revision 9
# speedup vs baseline: 2.2342x; 2.2342x over previous
"""Self-contained Trainium2 Bass kernel for sparse attention.

Sharding: 8 cores = (image b, L-half). Each core receives its image's x0
ROTATED so its own 4096 rows come first (gather indices are remapped on
the host to match). The core computes LN+K/V for all 8192 rows, writes
packed bf16 [k|v] rows to DRAM scratch, then per 128-row tile gathers
2048 neighbor rows with dma_gather and runs attention + merge + MLP +
LN2 fully on-chip. No collectives.
"""
import numpy as np
import ml_dtypes

import concourse.bass as bass
import concourse.tile as tile
from concourse import bacc, library_config, mybir

F32 = mybir.dt.float32
BF16 = mybir.dt.bfloat16
I16 = mybir.dt.int16
I32 = mybir.dt.int32
AX = mybir.AxisListType
OP = mybir.AluOpType
AF = mybir.ActivationFunctionType
ts = bass.ts

L, C, NJ, NH, HD = 8192, 128, 16, 8, 16
LH = L // 2            # rows computed per core
NT_FULL = L // 128     # 64 k/v tiles
NT_HALF = LH // 128    # 32 attention tiles
EPS = 1e-5


NSWQ = int(__import__("os").environ.get("NSWQ", "4"))


def build_nc(nontrivial_ln1: bool, nontrivial_ln2: bool):
    nc = bacc.Bacc(None, target_bir_lowering=False, debug=False,
                   num_swdge_queues=NSWQ)

    x0f = nc.declare_dram_parameter("x0f", [L, C], F32, isOutput=False)
    gidx16 = nc.declare_dram_parameter("gidx16", [128, NT_HALF * 128], I16, isOutput=False)
    wnames = ["wm", "w1", "w2"]
    wparams = {n: nc.declare_dram_parameter(n, [C, C], BF16, isOutput=False) for n in wnames}
    wkvp = nc.declare_dram_parameter("wqkv", [C, 3 * C], BF16, isOutput=False)
    identp = nc.declare_dram_parameter("ident", [C, C], BF16, isOutput=False)
    if nontrivial_ln1:
        bqkv = nc.declare_dram_parameter("bqkv", [1, 3 * C], F32, isOutput=False)
    if nontrivial_ln2:
        g2b2 = nc.declare_dram_parameter("g2b2", [1, 2 * C], F32, isOutput=False)
    out = nc.declare_dram_parameter("out", [LH, C], F32, isOutput=True)

    with tile.TileContext(nc) as tc:
        with (
            tc.tile_pool(name="res", bufs=1) as res,
            tc.tile_pool(name="dram", bufs=1, space="DRAM") as dram,
        ):
            nc.gpsimd.load_library(library_config.mlp)
            kv_dram = dram.tile([L, 2 * C], BF16)
            x0_all = res.tile([128, NT_FULL * 128], F32)   # all x0 tiles (ours first)
            x0_res = x0_all  # our-half rows are tiles 0..NT_HALF-1
            q_res = res.tile([128, NT_HALF * 128], BF16)   # our-half q tiles
            m2_all = res.tile([128, NT_HALF * 128], F32)   # MLP outputs for batched LN2
            muA = res.tile([128, NT_FULL], F32)
            rstdA = res.tile([128, NT_FULL], F32)
            idx16_res = res.tile([128, NT_HALF * 128], I16)
            nc.sync.dma_start(idx16_res[:], gidx16[:])
            ident = res.tile([128, 128], BF16)
            wsb = {n: res.tile([C, C], BF16, name=f"w_{n}", tag=f"w_{n}") for n in wnames}
            wqkv_sb = res.tile([C, 3 * C], BF16)
            nc.sync.dma_start(wqkv_sb[:], wkvp[:])

            for n in wnames:
                nc.sync.dma_start(wsb[n][:], wparams[n][:])
            nc.sync.dma_start(ident[:], identp[:])
            if nontrivial_ln1:
                bqkv_sb = res.tile([1, 3 * C], F32)
                nc.sync.dma_start(bqkv_sb[:], bqkv[:])
            if nontrivial_ln2:
                g2b2_sb = res.tile([1, 2 * C], F32)
                nc.sync.dma_start(g2b2_sb[:], g2b2[:])

            # ---------------- Phase 1a: load x0 + batched LN1 stats ----------------
            NCH = 8                      # tiles per stats chunk
            with tc.tile_pool(name="pa", bufs=2) as pa:
                ssumA = res.tile([128, NT_FULL], F32)
                s2A = res.tile([128, NT_FULL], F32)
                varA = res.tile([128, NT_FULL], F32)
                stdA = res.tile([128, NT_FULL], F32)
                mu2A = res.tile([128, NT_FULL], F32)
                for ch in range(NT_FULL // NCH):
                    lo = ch * NCH * 128
                    cs = slice(ch * NCH, (ch + 1) * NCH)
                    nc.sync.dma_start(
                        x0_all[:, lo:lo + NCH * 128].rearrange("p (t c) -> p t c", t=NCH),
                        x0f[ch * NCH * 128:(ch + 1) * NCH * 128, :].rearrange(
                            "(t p) c -> p t c", p=128))
                    sqc = pa.tile([128, NCH * 128], F32, tag="sqch")
                    nc.scalar.activation(sqc[:], x0_all[:, lo:lo + NCH * 128], AF.Square)
                    nc.vector.tensor_reduce(
                        ssumA[:, cs],
                        x0_all[:, lo:lo + NCH * 128].rearrange("p (t c) -> p t c", t=NCH),
                        axis=AX.X, op=OP.add)
                    nc.vector.tensor_reduce(
                        s2A[:, cs],
                        sqc[:].rearrange("p (t c) -> p t c", t=NCH),
                        axis=AX.X, op=OP.add)
                    # finalize this chunk's mu/rstd now so phase 1b can start
                    # on chunk 0 without waiting for the whole stats pass
                    nc.vector.tensor_scalar_mul(muA[:, cs], ssumA[:, cs], 1.0 / C)
                    nc.vector.tensor_tensor(mu2A[:, cs], muA[:, cs], muA[:, cs], op=OP.mult)
                    nc.vector.tensor_scalar(
                        varA[:, cs], s2A[:, cs], scalar1=1.0 / C, scalar2=EPS,
                        op0=OP.mult, op1=OP.add)
                    nc.vector.tensor_tensor(varA[:, cs], varA[:, cs], mu2A[:, cs], op=OP.subtract)
                    nc.scalar.activation(stdA[:, cs], varA[:, cs], AF.Sqrt)
                    nc.vector.reciprocal(rstdA[:, cs], stdA[:, cs])

            # ---------------- Phase 1b: xn + K/V (+Q) projections ----------------
            with (
                tc.tile_pool(name="p1", bufs=3) as p1,
                tc.tile_pool(name="p1s", bufs=2) as p1s,
                tc.tile_pool(name="ps1", bufs=2, space="PSUM") as ps1,
            ):
                for t in range(NT_FULL):
                    ours = t < NT_HALF
                    x0t = x0_all[:, ts(t, 128)]
                    xn = p1.tile([128, 128], BF16, tag="xn")
                    nc.vector.tensor_scalar(
                        xn[:], x0t, scalar1=muA[:, t:t + 1], scalar2=rstdA[:, t:t + 1],
                        op0=OP.subtract, op1=OP.mult,
                    )

                    # transpose xn -> xnT (bf16)
                    xnT_ps = ps1.tile([128, 128], BF16, tag="xnT_ps")
                    xnT = p1.tile([128, 128], BF16, tag="xnT")
                    nc.tensor.transpose(xnT_ps[:], xn[:], ident[:])
                    nc.scalar.copy(xnT[:], xnT_ps[:])

                    # q/k/v projections in one matmul -> packed [k|v] bf16 rows;
                    # 8 tiles of rows accumulate in SBUF, written with one DMA.
                    if t % 8 == 0:
                        kvt8 = p1.tile([128, 8 * 2 * C], BF16, tag="kvt8")
                    kvt = kvt8[:, (t % 8) * 2 * C:(t % 8 + 1) * 2 * C]
                    if ours:
                        qkv_ps = ps1.tile([128, 3 * C], F32, tag="qkv_ps")
                        nc.tensor.matmul(qkv_ps[:], lhsT=xnT[:], rhs=wqkv_sb[:], start=True, stop=True)
                        kv_ps = qkv_ps[:, C:3 * C]
                        if nontrivial_ln1:
                            nc.vector.tensor_tensor(
                                q_res[:, ts(t, 128)], qkv_ps[:, 0:C],
                                bqkv_sb[:, 0:C].to_broadcast([128, C]), op=OP.add)
                        else:
                            nc.scalar.copy(q_res[:, ts(t, 128)], qkv_ps[:, 0:C])
                    else:
                        kv_ps2 = ps1.tile([128, 2 * C], F32, tag="kv_ps")
                        nc.tensor.matmul(kv_ps2[:], lhsT=xnT[:], rhs=wqkv_sb[:, C:3 * C], start=True, stop=True)
                        kv_ps = kv_ps2[:]
                    if nontrivial_ln1:
                        nc.vector.tensor_tensor(
                            kvt, kv_ps,
                            bqkv_sb[:, C:3 * C].to_broadcast([128, 2 * C]), op=OP.add)
                    elif t % 2 == 0:
                        nc.scalar.copy(kvt[:, 0:C], kv_ps[:, 0:C])
                        nc.vector.tensor_scalar_mul(kvt[:, C:2 * C], kv_ps[:, C:2 * C], 1.0)
                    else:
                        nc.vector.tensor_scalar_mul(kvt, kv_ps, 1.0)
                    if t % 8 == 7:
                        nc.sync.dma_start(
                            kv_dram[(t - 7) * 128:(t + 1) * 128, :].rearrange(
                                "(u p) x -> p u x", p=128),
                            kvt8[:].rearrange("p (u x) -> p u x", u=8))

            # ---------------- Phase 2: gather + attention + MLP ----------------
            with (
                tc.tile_pool(name="p2", bufs=6) as p2,
                tc.tile_pool(name="p2s", bufs=3) as p2s,
                tc.tile_pool(name="ps2", bufs=1, space="PSUM") as ps2,
            ):
                for t in range(NT_HALF):
                    # 2 gathers of 1024 idxs (HW crashes somewhere in
                    # (1024, 1536] idxs per op), striped across SWDGE queues
                    kvg = p2.tile([128, NJ * 2 * C], BF16, tag="kvg")
                    for h in range(2):
                        nc.gpsimd.dma_gather(
                            kvg[:, h * (NJ // 2) * 2 * C:(h + 1) * (NJ // 2) * 2 * C]
                            .rearrange("p (j x) -> p j x", j=NJ // 2),
                            kv_dram[:],
                            idx16_res[:, t * 128 + h * 64:t * 128 + (h + 1) * 64],
                            NJ * 64,           # num_idxs
                            NJ * 64,           # num_idxs_reg
                            2 * C,             # elem_size (bf16 elements)
                            queue_num=(2 * t + h) % NSWQ,
                        )
                    kvg_j = kvg[:].rearrange("p (j x) -> p j x", j=NJ)

                    # qk = sum_d q*kg per (j, head)
                    prod = p2s.tile([128, NJ * C], BF16, tag="prod")
                    qk = p2s.tile([128, NJ * NH], F32, tag="qk")
                    nc.vector.tensor_tensor(
                        prod[:].rearrange("p (j c) -> p j c", j=NJ),
                        q_res[:, ts(t, 128)].unsqueeze(1).to_broadcast([128, NJ, C]),
                        kvg_j[:, :, 0:C],
                        op=OP.mult,
                    )
                    tr8 = p2s.tile([128, NJ * NH * 8], BF16, tag="tr8")
                    tr4 = p2s.tile([128, NJ * NH * 4], BF16, tag="tr4")
                    tr2 = p2s.tile([128, NJ * NH * 2], BF16, tag="tr2")
                    p4d = prod[:].rearrange("p (j h d) -> p j h d", j=NJ, h=NH)
                    t8 = tr8[:].rearrange("p (j h d) -> p j h d", j=NJ, h=NH)
                    t4 = tr4[:].rearrange("p (j h d) -> p j h d", j=NJ, h=NH)
                    t2 = tr2[:].rearrange("p (j h d) -> p j h d", j=NJ, h=NH)
                    nc.vector.tensor_tensor(t8, p4d[:, :, :, 0:8], p4d[:, :, :, 8:16], op=OP.add)
                    nc.vector.tensor_tensor(t4, t8[:, :, :, 0:4], t8[:, :, :, 4:8], op=OP.add)
                    nc.vector.tensor_tensor(t2, t4[:, :, :, 0:2], t4[:, :, :, 2:4], op=OP.add)
                    nc.vector.tensor_tensor(
                        qk[:].rearrange("p (j h) -> p j h", j=NJ, h=NH).unsqueeze(3),
                        t2[:, :, :, 0:1], t2[:, :, :, 1:2], op=OP.add)
                    # softmax over j (no max subtraction; |qk| <~ 6)
                    E = p2s.tile([128, NJ * NH], BF16, tag="E")
                    sE = p2s.tile([128, NH], F32, tag="sE")
                    rec = p2s.tile([128, NH], F32, tag="rec")
                    A = p2s.tile([128, NJ * NH], BF16, tag="A")
                    nc.scalar.activation(E[:], qk[:], AF.Exp)
                    nc.vector.tensor_reduce(
                        sE[:], E[:].rearrange("p (j h) -> p h j", j=NJ), axis=AX.X, op=OP.add
                    )
                    nc.vector.reciprocal(rec[:], sE[:])
                    nc.vector.tensor_tensor(
                        A[:].rearrange("p (j h) -> p j h", j=NJ),
                        E[:].rearrange("p (j h) -> p j h", j=NJ),
                        rec[:].unsqueeze(1).to_broadcast([128, NJ, NH]),
                        op=OP.mult,
                    )
                    # att = sum_j A * vg  (A broadcast over d fused into the multiply)
                    prod2 = p2s.tile([128, NJ * C], BF16, tag="prod2")
                    att = p2s.tile([128, C], BF16, tag="att")
                    nc.vector.tensor_tensor(
                        prod2[:].rearrange("p (j h d) -> p j h d", j=NJ, h=NH),
                        kvg_j[:, :, C:2 * C].rearrange("p j (h d) -> p j h d", h=NH),
                        A[:].rearrange("p (j h) -> p j h", j=NJ).unsqueeze(3).to_broadcast([128, NJ, NH, HD]),
                        op=OP.mult,
                    )
                    av8 = p2s.tile([128, 8 * C], BF16, tag="av8")
                    av4 = p2s.tile([128, 4 * C], BF16, tag="av4")
                    av2 = p2s.tile([128, 2 * C], BF16, tag="av2")
                    nc.vector.tensor_tensor(av8[:], prod2[:, 0:8 * C], prod2[:, 8 * C:16 * C], op=OP.add)
                    nc.vector.tensor_tensor(av4[:], av8[:, 0:4 * C], av8[:, 4 * C:8 * C], op=OP.add)
                    nc.vector.tensor_tensor(av2[:], av4[:, 0:2 * C], av4[:, 2 * C:4 * C], op=OP.add)
                    nc.vector.tensor_tensor(att[:], av2[:, 0:C], av2[:, C:2 * C], op=OP.add)

                    # merge: qv = att @ Wm.T ; message = x0 + qv
                    attT_ps = ps2.tile([128, 128], BF16, tag="attT_ps")
                    attT = p2s.tile([128, 128], BF16, tag="attT")
                    nc.tensor.transpose(attT_ps[:], att[:], ident[:])
                    nc.scalar.copy(attT[:], attT_ps[:])
                    qv_ps = ps2.tile([128, 128], F32, tag="qv_ps")
                    nc.tensor.matmul(qv_ps[:], lhsT=attT[:], rhs=wsb["wm"][:], start=True, stop=True)
                    msg = p2s.tile([128, 128], BF16, tag="msg")
                    nc.vector.tensor_tensor(msg[:], x0_res[:, ts(t, 128)], qv_ps[:], op=OP.add)

                    # mlp
                    msgT_ps = ps2.tile([128, 128], BF16, tag="msgT_ps")
                    msgT = p2s.tile([128, 128], BF16, tag="msgT")
                    nc.tensor.transpose(msgT_ps[:], msg[:], ident[:])
                    nc.scalar.copy(msgT[:], msgT_ps[:])
                    m1_ps = ps2.tile([128, 128], F32, tag="m1_ps")
                    nc.tensor.matmul(m1_ps[:], lhsT=msgT[:], rhs=wsb["w1"][:], start=True, stop=True)
                    m1 = p2s.tile([128, 128], BF16, tag="m1")
                    nc.scalar.activation(m1[:], m1_ps[:], AF.Relu)
                    m1T_ps = ps2.tile([128, 128], BF16, tag="m1T_ps")
                    m1T = p2s.tile([128, 128], BF16, tag="m1T")
                    nc.tensor.transpose(m1T_ps[:], m1[:], ident[:])
                    nc.scalar.copy(m1T[:], m1T_ps[:])
                    m2_ps = ps2.tile([128, 128], F32, tag="m2_ps")
                    nc.tensor.matmul(m2_ps[:], lhsT=m1T[:], rhs=wsb["w2"][:], start=True, stop=True)
                    nc.scalar.activation(m2_all[:, ts(t, 128)], m2_ps[:], AF.Copy)

                    # ---- LN2 + residual, finalized per 8-tile chunk so it
                    # overlaps the gather stream instead of trailing it ----
                    FCH = 8
                    if t % FCH == FCH - 1:
                        c0 = t - (FCH - 1)
                        m2c = m2_all[:, c0 * 128:(t + 1) * 128]
                        sqf = p2s.tile([128, FCH * 128], F32, tag="sqf")
                        ssumf = p2s.tile([128, FCH], F32, tag="ssumf")
                        s2f = p2s.tile([128, FCH], F32, tag="s2f")
                        muf = p2s.tile([128, FCH], F32, tag="muf")
                        mu2f = p2s.tile([128, FCH], F32, tag="mu2f")
                        varf = p2s.tile([128, FCH], F32, tag="varf")
                        stdf = p2s.tile([128, FCH], F32, tag="stdf")
                        rstdf = p2s.tile([128, FCH], F32, tag="rstdf")
                        nc.scalar.activation(sqf[:], m2c, AF.Square)
                        nc.vector.tensor_reduce(
                            ssumf[:], m2c.rearrange("p (t c) -> p t c", t=FCH),
                            axis=AX.X, op=OP.add)
                        nc.vector.tensor_reduce(
                            s2f[:], sqf[:].rearrange("p (t c) -> p t c", t=FCH),
                            axis=AX.X, op=OP.add)
                        nc.vector.tensor_scalar_mul(muf[:], ssumf[:], 1.0 / C)
                        nc.vector.tensor_tensor(mu2f[:], muf[:], muf[:], op=OP.mult)
                        nc.vector.tensor_scalar(
                            varf[:], s2f[:], scalar1=1.0 / C, scalar2=EPS,
                            op0=OP.mult, op1=OP.add)
                        nc.vector.tensor_tensor(varf[:], varf[:], mu2f[:], op=OP.subtract)
                        nc.scalar.activation(stdf[:], varf[:], AF.Sqrt)
                        nc.vector.reciprocal(rstdf[:], stdf[:])
                        for u in range(FCH):
                            tt = c0 + u
                            outt = p2s.tile([128, 128], F32, tag="outt")
                            nc.vector.tensor_scalar(
                                outt[:], m2_all[:, ts(tt, 128)],
                                scalar1=muf[:, u:u + 1], scalar2=rstdf[:, u:u + 1],
                                op0=OP.subtract, op1=OP.mult)
                            if nontrivial_ln2:
                                nc.vector.tensor_tensor(
                                    outt[:], outt[:], g2b2_sb[:, 0:C].to_broadcast([128, C]), op=OP.mult)
                                nc.vector.tensor_tensor(
                                    outt[:], outt[:], g2b2_sb[:, C:2 * C].to_broadcast([128, C]), op=OP.add)
                            nc.vector.tensor_tensor(outt[:], outt[:], x0_res[:, ts(tt, 128)], op=OP.add)
                            nc.sync.dma_start(out[ts(tt, 128), :], outt[:])

    nc.finalize()
    return nc


def prep_core_inputs(x0_img: np.ndarray, query_img: np.ndarray, half: int, w):
    """Host-side prep for one core. w: dict of raw f32 weights g1,b1,g2,b2,Wq..W2."""
    ofs = half * LH
    x0r = np.ascontiguousarray(np.roll(x0_img, -ofs, axis=0))
    lq = query_img[ofs:ofs + LH, :].astype(np.int64)
    lq = (lq - ofs) % L  # remap into rotated coordinates

    # dma_gather index layout: per tile t, linear index n = j*128+p gathers
    # kv[lq[t*128+p, j]] into out[p, j, :]. The idx buffer wraps n into 16
    # partitions (idx[n%16, n//16]) and is replicated 8x across 128 parts.
    gidx16 = np.zeros((128, NT_HALF * 128), np.int16)
    for t in range(NT_HALF):
        flat = lq[t * 128:(t + 1) * 128, :].T.reshape(-1)     # n = j*128+p
        wrap = flat.reshape(128, 16).T.astype(np.int16)       # [16, 128]
        gidx16[:, t * 128:(t + 1) * 128] = np.tile(wrap, (8, 1))
    bf = ml_dtypes.bfloat16
    g1 = w["g1"]
    m = {
        "x0f": x0r,
        "gidx16": gidx16,
        "ident": np.eye(C, dtype=np.float32).astype(bf),
        "wqkv": np.concatenate(
            [np.ascontiguousarray((0.25 * w["Wq"] * g1[None, :]).T),
             np.ascontiguousarray((w["Wk"] * g1[None, :]).T),
             np.ascontiguousarray((w["Wv"] * g1[None, :]).T)], axis=1).astype(bf),
        "wm": np.ascontiguousarray(w["Wm"].T).astype(bf),
        "w1": np.ascontiguousarray(w["W1"].T).astype(bf),
        "w2": np.ascontiguousarray(w["W2"].T).astype(bf),
    }
    b1 = w["b1"]
    nontrivial_ln1 = bool(np.any(b1 != 0.0))
    if nontrivial_ln1:
        m["bqkv"] = np.concatenate(
            [0.25 * (w["Wq"] @ b1), w["Wk"] @ b1, w["Wv"] @ b1]
        ).reshape(1, 3 * C).astype(np.float32)
    nontrivial_ln2 = bool(np.any(w["g2"] != 1.0) or np.any(w["b2"] != 0.0))
    if nontrivial_ln2:
        m["g2b2"] = np.concatenate([w["g2"], w["b2"]]).reshape(1, 2 * C).astype(np.float32)
    return m, nontrivial_ln1, nontrivial_ln2


def kernel(**inputs):
    from concourse.bass_utils import run_bass_kernel_spmd

    x0 = np.asarray(inputs["x0"], np.float32)
    query = np.asarray(inputs["query"])
    w = {k: np.asarray(inputs[k], np.float32)
         for k in ["Wq", "Wk", "Wv", "Wm", "W1", "W2", "g1", "b1", "g2", "b2"]}
    B = x0.shape[0]

    in_maps = []
    nt1 = nt2 = False
    for c in range(8):
        b, half = c // 2, c % 2
        m, nt1, nt2 = prep_core_inputs(x0[b], np.asarray(query[b]), half, w)
        in_maps.append(m)

    nc = build_nc(nt1, nt2)
    res = run_bass_kernel_spmd(nc, in_maps, core_ids=list(range(8)))

    outp = np.empty((B, L, C), np.float32)
    for c in range(8):
        b, half = c // 2, c % 2
        outp[b, half * LH:(half + 1) * LH, :] = res.results[c]["out"]
    return outp



# revision 22
# speedup vs baseline: 2.6400x; 1.1816x over previous
"""Self-contained Trainium2 Bass kernel for sparse attention.

Sharding: 8 cores = (image b, L-half). Each core receives its image's x0
ROTATED so its own 4096 rows come first (gather indices are remapped on
the host to match). The core computes LN+K/V for all 8192 rows, writes
packed bf16 [k|v] rows to DRAM scratch, then per 128-row tile gathers
2048 neighbor rows with dma_gather and runs attention + merge + MLP +
LN2 fully on-chip. No collectives.
"""
import numpy as np
import ml_dtypes

import concourse.bass as bass
import concourse.tile as tile
from concourse import bacc, library_config, mybir

F32 = mybir.dt.float32
BF16 = mybir.dt.bfloat16
I16 = mybir.dt.int16
I32 = mybir.dt.int32
AX = mybir.AxisListType
OP = mybir.AluOpType
AF = mybir.ActivationFunctionType
ts = bass.ts

L, C, NJ, NH, HD = 8192, 128, 16, 8, 16
LH = L // 2            # rows computed per core
NT_FULL = L // 128     # 64 k/v tiles
NT_HALF = LH // 128    # 32 attention tiles
EPS = 1e-5


NSWQ = int(__import__("os").environ.get("NSWQ", "4"))


def build_nc(nontrivial_ln1: bool, nontrivial_ln2: bool):
    nc = bacc.Bacc(None, target_bir_lowering=False, debug=False,
                   num_swdge_queues=NSWQ)

    x0f = nc.declare_dram_parameter("x0f", [L, C], F32, isOutput=False)
    gidx16 = nc.declare_dram_parameter("gidx16", [128, NT_HALF * 128], I16, isOutput=False)
    wnames = ["wm", "w1", "w2"]
    wparams = {n: nc.declare_dram_parameter(n, [C, C], BF16, isOutput=False) for n in wnames}
    wkvp = nc.declare_dram_parameter("wqkv", [C, 3 * C], BF16, isOutput=False)
    identp = nc.declare_dram_parameter("ident", [C, C], BF16, isOutput=False)
    if nontrivial_ln1:
        bqkv = nc.declare_dram_parameter("bqkv", [1, 3 * C], F32, isOutput=False)
    if nontrivial_ln2:
        g2b2 = nc.declare_dram_parameter("g2b2", [1, 2 * C], F32, isOutput=False)
    out = nc.declare_dram_parameter("out", [LH, C], F32, isOutput=True)

    with tile.TileContext(nc) as tc:
        with (
            tc.tile_pool(name="res", bufs=1) as res,
            tc.tile_pool(name="dram", bufs=1, space="DRAM") as dram,
        ):
            nc.gpsimd.load_library(library_config.mlp)
            kv_dram = dram.tile([L, 2 * C], BF16)
            x0_all = res.tile([128, NT_FULL * 128], F32)   # all x0 tiles (ours first)
            x0_res = x0_all  # our-half rows are tiles 0..NT_HALF-1
            q_res = res.tile([128, NT_HALF * 128], BF16)   # our-half q tiles
            m2_all = res.tile([128, NT_HALF * 128], F32)   # MLP outputs for batched LN2
            muA = res.tile([128, NT_FULL], F32)
            rstdA = res.tile([128, NT_FULL], F32)
            nbiasA = res.tile([128, NT_FULL], F32)
            idx16_res = res.tile([128, NT_HALF * 128], I16)
            nc.sync.dma_start(idx16_res[:], gidx16[:])
            ident = res.tile([128, 128], BF16)
            wsb = {n: res.tile([C, C], BF16, name=f"w_{n}", tag=f"w_{n}") for n in wnames}
            wqkv_sb = res.tile([C, 3 * C], BF16)
            nc.sync.dma_start(wqkv_sb[:], wkvp[:])

            for n in wnames:
                nc.sync.dma_start(wsb[n][:], wparams[n][:])
            nc.sync.dma_start(ident[:], identp[:])
            if nontrivial_ln1:
                bqkv_sb = res.tile([1, 3 * C], F32)
                nc.sync.dma_start(bqkv_sb[:], bqkv[:])
            if nontrivial_ln2:
                g2b2_sb = res.tile([1, 2 * C], F32)
                nc.sync.dma_start(g2b2_sb[:], g2b2[:])

            # ---------------- Phase 1a: load x0 + batched LN1 stats ----------------
            NCH = 8                      # tiles per stats chunk
            with tc.tile_pool(name="pa", bufs=2) as pa:
                ssumA = res.tile([128, NT_FULL], F32)
                s2A = res.tile([128, NT_FULL], F32)
                varA = res.tile([128, NT_FULL], F32)
                stdA = res.tile([128, NT_FULL], F32)
                mu2A = res.tile([128, NT_FULL], F32)
                for ch in range(NT_FULL // NCH):
                    lo = ch * NCH * 128
                    cs = slice(ch * NCH, (ch + 1) * NCH)
                    nc.sync.dma_start(
                        x0_all[:, lo:lo + NCH * 128].rearrange("p (t c) -> p t c", t=NCH),
                        x0f[ch * NCH * 128:(ch + 1) * NCH * 128, :].rearrange(
                            "(t p) c -> p t c", p=128))
                    sqc = pa.tile([128, NCH * 128], F32, tag="sqch")
                    nc.scalar.activation(sqc[:], x0_all[:, lo:lo + NCH * 128], AF.Square)
                    nc.vector.tensor_reduce(
                        ssumA[:, cs],
                        x0_all[:, lo:lo + NCH * 128].rearrange("p (t c) -> p t c", t=NCH),
                        axis=AX.X, op=OP.add)
                    nc.vector.tensor_reduce(
                        s2A[:, cs],
                        sqc[:].rearrange("p (t c) -> p t c", t=NCH),
                        axis=AX.X, op=OP.add)
                    # finalize this chunk's mu/rstd now so phase 1b can start
                    # on chunk 0 without waiting for the whole stats pass
                    nc.vector.tensor_scalar_mul(muA[:, cs], ssumA[:, cs], 1.0 / C)
                    nc.vector.tensor_tensor(mu2A[:, cs], muA[:, cs], muA[:, cs], op=OP.mult)
                    nc.vector.tensor_scalar(
                        varA[:, cs], s2A[:, cs], scalar1=1.0 / C, scalar2=EPS,
                        op0=OP.mult, op1=OP.add)
                    nc.vector.tensor_tensor(varA[:, cs], varA[:, cs], mu2A[:, cs], op=OP.subtract)
                    nc.scalar.activation(stdA[:, cs], varA[:, cs], AF.Sqrt)
                    nc.vector.reciprocal(rstdA[:, cs], stdA[:, cs])
                    nc.vector.tensor_scalar_mul(nbiasA[:, cs], muA[:, cs], -1.0)
                    nc.vector.tensor_tensor(
                        nbiasA[:, cs], nbiasA[:, cs], rstdA[:, cs], op=OP.mult)

            # ---------------- Phase 1b: xn + K/V (+Q) projections ----------------
            with (
                tc.tile_pool(name="p1", bufs=3) as p1,
                tc.tile_pool(name="p1s", bufs=2) as p1s,
                tc.tile_pool(name="ps1", bufs=2, space="PSUM") as ps1,
            ):
                for t in range(NT_FULL):
                    ours = t < NT_HALF
                    x0t = x0_all[:, ts(t, 128)]
                    xn = p1.tile([128, 128], BF16, tag="xn")
                    # (x0 - mu) * rstd as act(x0*rstd + (-mu*rstd)) on Scalar
                    nc.scalar.activation(
                        xn[:], x0t, AF.Identity,
                        bias=nbiasA[:, t:t + 1], scale=rstdA[:, t:t + 1])

                    # transpose xn -> xnT (bf16)
                    xnT_ps = ps1.tile([128, 128], BF16, tag="xnT_ps")
                    xnT = p1.tile([128, 128], BF16, tag="xnT")
                    nc.tensor.transpose(xnT_ps[:], xn[:], ident[:])
                    nc.vector.tensor_scalar_mul(xnT[:], xnT_ps[:], 1.0)

                    # q/k/v projections in one matmul -> packed [k|v] bf16 rows;
                    # 8 tiles of rows accumulate in SBUF, written with one DMA.
                    if t % 8 == 0:
                        kvt8 = p1.tile([128, 8 * 2 * C], BF16, tag="kvt8")
                    kvt = kvt8[:, (t % 8) * 2 * C:(t % 8 + 1) * 2 * C]
                    if ours:
                        qkv_ps = ps1.tile([128, 3 * C], F32, tag="qkv_ps")
                        nc.tensor.matmul(qkv_ps[:], lhsT=xnT[:], rhs=wqkv_sb[:], start=True, stop=True)
                        kv_ps = qkv_ps[:, C:3 * C]
                        if nontrivial_ln1:
                            nc.vector.tensor_tensor(
                                q_res[:, ts(t, 128)], qkv_ps[:, 0:C],
                                bqkv_sb[:, 0:C].to_broadcast([128, C]), op=OP.add)
                        else:
                            nc.scalar.copy(q_res[:, ts(t, 128)], qkv_ps[:, 0:C])
                    else:
                        kv_ps2 = ps1.tile([128, 2 * C], F32, tag="kv_ps")
                        nc.tensor.matmul(kv_ps2[:], lhsT=xnT[:], rhs=wqkv_sb[:, C:3 * C], start=True, stop=True)
                        kv_ps = kv_ps2[:]
                    if nontrivial_ln1:
                        nc.vector.tensor_tensor(
                            kvt, kv_ps,
                            bqkv_sb[:, C:3 * C].to_broadcast([128, 2 * C]), op=OP.add)
                    elif t % 2 == 0:
                        nc.scalar.copy(kvt[:, 0:C], kv_ps[:, 0:C])
                        nc.vector.tensor_scalar_mul(kvt[:, C:2 * C], kv_ps[:, C:2 * C], 1.0)
                    else:
                        nc.vector.tensor_scalar_mul(kvt, kv_ps, 1.0)
                    if t % 8 == 7:
                        nc.sync.dma_start(
                            kv_dram[(t - 7) * 128:(t + 1) * 128, :].rearrange(
                                "(u p) x -> p u x", p=128),
                            kvt8[:].rearrange("p (u x) -> p u x", u=8))

            # ---------------- Phase 2: gather + attention + MLP ----------------
            with (
                tc.tile_pool(name="p2", bufs=6) as p2,
                tc.tile_pool(name="p2s", bufs=3) as p2s,
                tc.tile_pool(name="ps2", bufs=1, space="PSUM") as ps2,
            ):
                for t in range(NT_HALF):
                    # 2 gathers of 1024 idxs (HW crashes somewhere in
                    # (1024, 1536] idxs per op), striped across SWDGE queues
                    kvg = p2.tile([128, NJ * 2 * C], BF16, tag="kvg")
                    for h in range(2):
                        nc.gpsimd.dma_gather(
                            kvg[:, h * (NJ // 2) * 2 * C:(h + 1) * (NJ // 2) * 2 * C]
                            .rearrange("p (j x) -> p j x", j=NJ // 2),
                            kv_dram[:],
                            idx16_res[:, t * 128 + h * 64:t * 128 + (h + 1) * 64],
                            NJ * 64,           # num_idxs
                            NJ * 64,           # num_idxs_reg
                            2 * C,             # elem_size (bf16 elements)
                            queue_num=(2 * t + h) % NSWQ,
                        )
                    kvg_j = kvg[:].rearrange("p (j x) -> p j x", j=NJ)

                    # qk = sum_d q*kg per (j, head)
                    prod = p2s.tile([128, NJ * C], BF16, tag="prod")
                    qk = p2s.tile([128, NJ * NH], F32, tag="qk")
                    nc.vector.tensor_tensor(
                        prod[:].rearrange("p (j c) -> p j c", j=NJ),
                        q_res[:, ts(t, 128)].unsqueeze(1).to_broadcast([128, NJ, C]),
                        kvg_j[:, :, 0:C],
                        op=OP.mult,
                    )
                    tr8 = p2s.tile([128, NJ * NH * 8], BF16, tag="tr8")
                    tr4 = p2s.tile([128, NJ * NH * 4], BF16, tag="tr4")
                    tr2 = p2s.tile([128, NJ * NH * 2], BF16, tag="tr2")
                    p4d = prod[:].rearrange("p (j h d) -> p j h d", j=NJ, h=NH)
                    t8 = tr8[:].rearrange("p (j h d) -> p j h d", j=NJ, h=NH)
                    t4 = tr4[:].rearrange("p (j h d) -> p j h d", j=NJ, h=NH)
                    t2 = tr2[:].rearrange("p (j h d) -> p j h d", j=NJ, h=NH)
                    nc.vector.tensor_tensor(t8, p4d[:, :, :, 0:8], p4d[:, :, :, 8:16], op=OP.add)
                    nc.vector.tensor_tensor(t4, t8[:, :, :, 0:4], t8[:, :, :, 4:8], op=OP.add)
                    nc.vector.tensor_tensor(t2, t4[:, :, :, 0:2], t4[:, :, :, 2:4], op=OP.add)
                    nc.vector.tensor_tensor(
                        qk[:].rearrange("p (j h) -> p j h", j=NJ, h=NH).unsqueeze(3),
                        t2[:, :, :, 0:1], t2[:, :, :, 1:2], op=OP.add)
                    # softmax over j (no max subtraction; |qk| <~ 6).
                    # E is expanded over d on the Scalar engine so the AV
                    # product stays fully packed (2x DVE mode); the 1/sum
                    # normalization is applied after the j-sum instead.
                    E = p2s.tile([128, NJ * NH], BF16, tag="E")
                    sE = p2s.tile([128, NH], F32, tag="sE")
                    rec = p2s.tile([128, NH], F32, tag="rec")
                    Eex = p2s.tile([128, NJ * C], BF16, tag="Eex")
                    nc.scalar.activation(E[:], qk[:], AF.Exp)
                    nc.vector.tensor_reduce(
                        sE[:], E[:].rearrange("p (j h) -> p h j", j=NJ), axis=AX.X, op=OP.add
                    )
                    nc.vector.reciprocal(rec[:], sE[:])
                    nc.scalar.activation(
                        Eex[:].rearrange("p (j h d) -> p j h d", j=NJ, h=NH),
                        qk[:].rearrange("p (j h) -> p j h", j=NJ)
                        .unsqueeze(3).to_broadcast([128, NJ, NH, HD]),
                        AF.Exp)
                    # att_u = sum_j E * vg ; att = att_u * rec
                    prod2 = p2s.tile([128, NJ * C], BF16, tag="prod2")
                    att = p2s.tile([128, C], BF16, tag="att")
                    nc.vector.tensor_tensor(
                        prod2[:].rearrange("p (j x) -> p j x", j=NJ),
                        kvg_j[:, :, C:2 * C],
                        Eex[:].rearrange("p (j x) -> p j x", j=NJ),
                        op=OP.mult,
                    )
                    av8 = p2s.tile([128, 8 * C], BF16, tag="av8")
                    av4 = p2s.tile([128, 4 * C], BF16, tag="av4")
                    av2 = p2s.tile([128, 2 * C], BF16, tag="av2")
                    nc.vector.tensor_tensor(av8[:], prod2[:, 0:8 * C], prod2[:, 8 * C:16 * C], op=OP.add)
                    nc.vector.tensor_tensor(av4[:], av8[:, 0:4 * C], av8[:, 4 * C:8 * C], op=OP.add)
                    nc.vector.tensor_tensor(av2[:], av4[:, 0:2 * C], av4[:, 2 * C:4 * C], op=OP.add)
                    attu = p2s.tile([128, C], BF16, tag="attu")
                    nc.vector.tensor_tensor(attu[:], av2[:, 0:C], av2[:, C:2 * C], op=OP.add)
                    nc.vector.tensor_tensor(
                        att[:].rearrange("p (h d) -> p h d", h=NH),
                        attu[:].rearrange("p (h d) -> p h d", h=NH),
                        rec[:].unsqueeze(2).to_broadcast([128, NH, HD]),
                        op=OP.mult,
                    )

                    # merge: qv = att @ Wm.T ; message = x0 + qv
                    attT_ps = ps2.tile([128, 128], BF16, tag="attT_ps")
                    attT = p2s.tile([128, 128], BF16, tag="attT")
                    nc.tensor.transpose(attT_ps[:], att[:], ident[:])
                    nc.scalar.copy(attT[:], attT_ps[:])
                    qv_ps = ps2.tile([128, 128], F32, tag="qv_ps")
                    nc.tensor.matmul(qv_ps[:], lhsT=attT[:], rhs=wsb["wm"][:], start=True, stop=True)
                    msg = p2s.tile([128, 128], BF16, tag="msg")
                    nc.vector.tensor_tensor(msg[:], x0_res[:, ts(t, 128)], qv_ps[:], op=OP.add)

                    # mlp
                    msgT_ps = ps2.tile([128, 128], BF16, tag="msgT_ps")
                    msgT = p2s.tile([128, 128], BF16, tag="msgT")
                    nc.tensor.transpose(msgT_ps[:], msg[:], ident[:])
                    nc.scalar.copy(msgT[:], msgT_ps[:])
                    m1_ps = ps2.tile([128, 128], F32, tag="m1_ps")
                    nc.tensor.matmul(m1_ps[:], lhsT=msgT[:], rhs=wsb["w1"][:], start=True, stop=True)
                    m1 = p2s.tile([128, 128], BF16, tag="m1")
                    nc.scalar.activation(m1[:], m1_ps[:], AF.Relu)
                    m1T_ps = ps2.tile([128, 128], BF16, tag="m1T_ps")
                    m1T = p2s.tile([128, 128], BF16, tag="m1T")
                    nc.tensor.transpose(m1T_ps[:], m1[:], ident[:])
                    nc.scalar.copy(m1T[:], m1T_ps[:])
                    m2_ps = ps2.tile([128, 128], F32, tag="m2_ps")
                    nc.tensor.matmul(m2_ps[:], lhsT=m1T[:], rhs=wsb["w2"][:], start=True, stop=True)
                    nc.scalar.activation(m2_all[:, ts(t, 128)], m2_ps[:], AF.Copy)

                    # ---- LN2 + residual, finalized per 8-tile chunk so it
                    # overlaps the gather stream instead of trailing it ----
                    FCH = 8
                    if t % FCH == FCH - 1:
                        c0 = t - (FCH - 1)
                        m2c = m2_all[:, c0 * 128:(t + 1) * 128]
                        sqf = p2s.tile([128, FCH * 128], F32, tag="sqf")
                        ssumf = p2s.tile([128, FCH], F32, tag="ssumf")
                        s2f = p2s.tile([128, FCH], F32, tag="s2f")
                        muf = p2s.tile([128, FCH], F32, tag="muf")
                        mu2f = p2s.tile([128, FCH], F32, tag="mu2f")
                        varf = p2s.tile([128, FCH], F32, tag="varf")
                        stdf = p2s.tile([128, FCH], F32, tag="stdf")
                        rstdf = p2s.tile([128, FCH], F32, tag="rstdf")
                        nc.scalar.activation(sqf[:], m2c, AF.Square)
                        nc.vector.tensor_reduce(
                            ssumf[:], m2c.rearrange("p (t c) -> p t c", t=FCH),
                            axis=AX.X, op=OP.add)
                        nc.vector.tensor_reduce(
                            s2f[:], sqf[:].rearrange("p (t c) -> p t c", t=FCH),
                            axis=AX.X, op=OP.add)
                        nbias = p2s.tile([128, FCH], F32, tag="nbias")
                        nc.vector.tensor_scalar_mul(muf[:], ssumf[:], 1.0 / C)
                        nc.vector.tensor_tensor(mu2f[:], muf[:], muf[:], op=OP.mult)
                        nc.vector.tensor_scalar(
                            varf[:], s2f[:], scalar1=1.0 / C, scalar2=EPS,
                            op0=OP.mult, op1=OP.add)
                        nc.vector.tensor_tensor(varf[:], varf[:], mu2f[:], op=OP.subtract)
                        nc.scalar.activation(stdf[:], varf[:], AF.Sqrt)
                        nc.vector.reciprocal(rstdf[:], stdf[:])
                        nc.vector.tensor_scalar_mul(nbias[:], muf[:], -1.0)
                        nc.vector.tensor_tensor(nbias[:], nbias[:], rstdf[:], op=OP.mult)
                        for u in range(FCH):
                            tt = c0 + u
                            outt = p2s.tile([128, 128], F32, tag="outt")
                            # (m2 - mu) * rstd on the Scalar engine
                            nc.scalar.activation(
                                outt[:], m2_all[:, ts(tt, 128)], AF.Identity,
                                bias=nbias[:, u:u + 1], scale=rstdf[:, u:u + 1])
                            if nontrivial_ln2:
                                nc.vector.tensor_tensor(
                                    outt[:], outt[:], g2b2_sb[:, 0:C].to_broadcast([128, C]), op=OP.mult)
                                nc.vector.tensor_tensor(
                                    outt[:], outt[:], g2b2_sb[:, C:2 * C].to_broadcast([128, C]), op=OP.add)
                            nc.vector.tensor_tensor(outt[:], outt[:], x0_res[:, ts(tt, 128)], op=OP.add)
                            nc.sync.dma_start(out[ts(tt, 128), :], outt[:])

    nc.finalize()
    return nc


def prep_core_inputs(x0_img: np.ndarray, query_img: np.ndarray, half: int, w):
    """Host-side prep for one core. w: dict of raw f32 weights g1,b1,g2,b2,Wq..W2."""
    ofs = half * LH
    x0r = np.ascontiguousarray(np.roll(x0_img, -ofs, axis=0))
    lq = query_img[ofs:ofs + LH, :].astype(np.int64)
    lq = (lq - ofs) % L  # remap into rotated coordinates

    # dma_gather index layout: per tile t, linear index n = j*128+p gathers
    # kv[lq[t*128+p, j]] into out[p, j, :]. The idx buffer wraps n into 16
    # partitions (idx[n%16, n//16]) and is replicated 8x across 128 parts.
    gidx16 = np.zeros((128, NT_HALF * 128), np.int16)
    for t in range(NT_HALF):
        flat = lq[t * 128:(t + 1) * 128, :].T.reshape(-1)     # n = j*128+p
        wrap = flat.reshape(128, 16).T.astype(np.int16)       # [16, 128]
        gidx16[:, t * 128:(t + 1) * 128] = np.tile(wrap, (8, 1))
    bf = ml_dtypes.bfloat16
    g1 = w["g1"]
    m = {
        "x0f": x0r,
        "gidx16": gidx16,
        "ident": np.eye(C, dtype=np.float32).astype(bf),
        "wqkv": np.concatenate(
            [np.ascontiguousarray((0.25 * w["Wq"] * g1[None, :]).T),
             np.ascontiguousarray((w["Wk"] * g1[None, :]).T),
             np.ascontiguousarray((w["Wv"] * g1[None, :]).T)], axis=1).astype(bf),
        "wm": np.ascontiguousarray(w["Wm"].T).astype(bf),
        "w1": np.ascontiguousarray(w["W1"].T).astype(bf),
        "w2": np.ascontiguousarray(w["W2"].T).astype(bf),
    }
    b1 = w["b1"]
    nontrivial_ln1 = bool(np.any(b1 != 0.0))
    if nontrivial_ln1:
        m["bqkv"] = np.concatenate(
            [0.25 * (w["Wq"] @ b1), w["Wk"] @ b1, w["Wv"] @ b1]
        ).reshape(1, 3 * C).astype(np.float32)
    nontrivial_ln2 = bool(np.any(w["g2"] != 1.0) or np.any(w["b2"] != 0.0))
    if nontrivial_ln2:
        m["g2b2"] = np.concatenate([w["g2"], w["b2"]]).reshape(1, 2 * C).astype(np.float32)
    return m, nontrivial_ln1, nontrivial_ln2


def kernel(**inputs):
    from concourse.bass_utils import run_bass_kernel_spmd

    x0 = np.asarray(inputs["x0"], np.float32)
    query = np.asarray(inputs["query"])
    w = {k: np.asarray(inputs[k], np.float32)
         for k in ["Wq", "Wk", "Wv", "Wm", "W1", "W2", "g1", "b1", "g2", "b2"]}
    B = x0.shape[0]

    in_maps = []
    nt1 = nt2 = False
    for c in range(8):
        b, half = c // 2, c % 2
        m, nt1, nt2 = prep_core_inputs(x0[b], np.asarray(query[b]), half, w)
        in_maps.append(m)

    nc = build_nc(nt1, nt2)
    res = run_bass_kernel_spmd(nc, in_maps, core_ids=list(range(8)))

    outp = np.empty((B, L, C), np.float32)
    for c in range(8):
        b, half = c // 2, c % 2
        outp[b, half * LH:(half + 1) * LH, :] = res.results[c]["out"]
    return outp

